# revision 13
# baseline (speedup 1.0000x reference)
"""DiT block kernel for 8 Trainium2 NeuronCores (Bass/Tile, SPMD).

Sharding: core c = 4*b + j handles batch b, tokens 512j..512j+512.
Each core computes LN1 + Q/K/V only for its own 512 tokens; K^T and V
are exchanged with two AllGathers inside the 4-core group, then each
core runs attention for its own queries against all 2048 keys (all 12
heads). The adaLN (scale_shift) first GEMM is column-sharded 8-ways
(one tiny AllToAll also routes each core its batch's row), the second
4-ways per group (one tiny AllGather).

Everything runs in the transposed layout [feature-partitions, tokens]:
per-feature modulation constants are per-partition scalars, per-token
LN stats are matmul-broadcast rows. The second GEMM of each MLP swaps
stationary/moving so its output is already transposed; the final output
is written feature-major and transposed on the host. Softmax
normalization is batched: one [12,512] reciprocal + one broadcast DMA.

All matmuls bf16 with fp32 PSUM accumulation; residual stream is fp32.
"""
import sys
sys.path.insert(0, "/opt/trn_rl_repo")

import numpy as np
import ml_dtypes

import concourse.bass as bass
import concourse.tile as tile
from concourse import bacc, mybir
from concourse.bass_utils import run_bass_kernel_spmd

P = 128
H = 768
NH = 12
HD = 64
B = 2
T = 2048
TOK = 512            # own tokens per core
KT6 = H // P         # 6 k-tiles over hidden
TT16 = T // P        # 16 token tiles over full batch
MT4 = TOK // P       # 4 token tiles over own tokens
NQ4 = T // TOK       # 4 token-quarter shards in a group
FF = 3072
FFT = FF // P        # 24
SS = 6 * H           # 4608
SSC = SS // 8        # 576  ss1 columns per core (8-way)
SSH = SS // 4        # 1152 ss2 columns per core (4-way in group)
SCALE = float(1.0 / np.sqrt(H))
EPS = 1e-5

BF = mybir.dt.bfloat16
F32 = mybir.dt.float32
AF = mybir.ActivationFunctionType
ALU = mybir.AluOpType

N_CORES = 8
STAGE = 9  # emit stages up to this number
SIM_SAFE = False
GROUPS4 = [[0, 1, 2, 3], [4, 5, 6, 7]]
GROUPS8 = [[0, 1, 2, 3, 4, 5, 6, 7]]


def _rep(dram_ap, times):
    """DRAM AP -> AP with a leading 0-stride axis reading it `times` times."""
    return bass.AP(tensor=dram_ap.tensor, offset=dram_ap.offset,
                   ap=[[0, times]] + [list(a) for a in dram_ap.ap])


def _emit(ctx, tc, io):
    nc = tc.nc

    const = ctx.enter_context(tc.tile_pool(name="const", bufs=1))
    psum_big = ctx.enter_context(tc.tile_pool(name="psum_big", bufs=4, space="PSUM"))
    psum_o = ctx.enter_context(tc.tile_pool(name="psum_o", bufs=3, space="PSUM"))
    dram = ctx.enter_context(tc.tile_pool(name="dram", bufs=12, space="DRAM"))
    wrk = ctx.enter_context(tc.tile_pool(name="wrk", bufs=6))
    small = ctx.enter_context(tc.tile_pool(name="small", bufs=8))
    wn = ctx.enter_context(tc.tile_pool(name="wn", bufs=6))
    wk = ctx.enter_context(tc.tile_pool(name="wk", bufs=14))
    eP = ctx.enter_context(tc.tile_pool(name="eP", bufs=4))

    # ---------- constants ----------
    ones_bf = const.tile([P, P], BF, name="ones_bf")
    nc.vector.memset(ones_bf[:], 1.0)
    eps_ap = const.tile([P, 1], F32, name="eps")
    nc.vector.memset(eps_ap[:], EPS)

    ln1g_c = const.tile([P, KT6], F32, name="ln1g")
    nc.sync.dma_start(ln1g_c[:], io["ln1g_c"][:])
    ln1b_c = const.tile([P, KT6], F32, name="ln1b")
    nc.sync.dma_start(ln1b_c[:], io["ln1b_c"][:])
    ln2g_c = const.tile([P, KT6], F32, name="ln2g")
    nc.sync.dma_start(ln2g_c[:], io["ln2g_c"][:])
    ln2b_c = const.tile([P, KT6], F32, name="ln2b")
    nc.sync.dma_start(ln2b_c[:], io["ln2b_c"][:])

    # ---------- stage 1: scale_shift (adaLN) ----------
    # ss1 (8-way col shard): [2, 576] = t2T.T @ ss1_slice  (M=2: both batches)
    tT_sb = const.tile([P, KT6, 2], BF, name="tT")
    nc.sync.dma_start(tT_sb[:], io["t2T"].rearrange("(k p) b -> p k b", p=P))
    silu_sl = small.tile([2, SSC], BF, name="silu_sl", bufs=1)
    for n0, nsz in [(0, 512), (512, 64)]:
        ps = psum_big.tile([P, 512], F32, name="pbig")[0:2, 0:nsz]
        for k in range(KT6):
            w_t = wn.tile([P, 512], BF, name="wn")[:, 0:nsz]
            nc.sync.dma_start(w_t, io["ss1s"][P * k:P * (k + 1), n0:n0 + nsz])
            nc.tensor.matmul(ps, tT_sb[:, k, :], w_t,
                             start=(k == 0), stop=(k == KT6 - 1))
        sig = wrk.tile([P, 512], F32, name="w512")[0:2, 0:nsz]
        nc.scalar.activation(sig, ps, AF.Sigmoid)
        nc.vector.tensor_mul(silu_sl[:, n0:n0 + nsz], ps, sig)

    # AllToAll over all 8 so every core ends with the full 4608-wide silu
    # row of ITS batch: send row (r//4) of my shard to rank r.
    silu_sm = dram.tile([2, SSC], BF)
    nc.sync.dma_start(silu_sm[:], silu_sl[:])
    silu_in = dram.tile([8, SSC], BF)
    nc.sync.dma_start(silu_in.rearrange("(b r) g -> b r g", b=2),
                      bass.AP(tensor=silu_sm.tensor, offset=silu_sm.offset,
                              ap=[[SSC, 2], [0, 4], [1, SSC]]))
    silu_out = dram.tile([8, SSC], BF)
    nc.gpsimd.collective_compute(
        "AllToAll", ALU.bypass, replica_groups=GROUPS8,
        ins=[silu_in.opt()], outs=[silu_out.opt()],
    )
    silu_cols = const.tile([P, SS // P], BF, name="silu_cols")
    nc.sync.dma_start(silu_cols[:], silu_out.rearrange("r g -> (r g)")
                      .rearrange("(k p) -> p k", p=P))

    # ss2 (4-way col shard in group): [1, 1152] = silu_my @ ss2_slice
    temb_sl = small.tile([1, SSH], F32, name="temb_sl", bufs=1)
    for n0, nsz in [(0, 512), (512, 512), (1024, 128)]:
        ps = psum_big.tile([P, 512], F32, name="pbig")[0:1, 0:nsz]
        for k in range(SS // P):    # 36
            w_t = wn.tile([P, 512], BF, name="wn")[:, 0:nsz]
            nc.sync.dma_start(w_t, io["ss2s"][P * k:P * (k + 1), n0:n0 + nsz])
            nc.tensor.matmul(ps, silu_cols[:, k:k + 1], w_t,
                             start=(k == 0), stop=(k == SS // P - 1))
        nc.vector.tensor_copy(temb_sl[:, n0:n0 + nsz], ps)

    temb_sm = dram.tile([1, SSH], F32)
    nc.sync.dma_start(temb_sm[:], temb_sl[:])
    temb_out = dram.tile([4, SSH], F32)
    nc.gpsimd.collective_compute(
        "AllGather", ALU.bypass, replica_groups=GROUPS4,
        ins=[temb_sm.opt()], outs=[temb_out.opt()],
    )
    temb_flat = temb_out.rearrange("r g -> (r g)")  # [4608] my batch
    # six sections -> column layout [128, 6] each
    sec = {}
    for i, name in enumerate(["g1", "be1", "a1", "g2", "be2", "a2"]):
        t = const.tile([P, KT6], F32, name=f"sec_{name}")
        nc.sync.dma_start(t[:], temb_flat[H * i:H * (i + 1)]
                          .rearrange("(k p) -> p k", p=P))
        sec[name] = t
    G1c = const.tile([P, KT6], F32, name="G1c")
    nc.vector.tensor_mul(G1c[:], sec["g1"][:], ln1g_c[:])
    B1c = const.tile([P, KT6], F32, name="B1c")
    nc.vector.tensor_mul(B1c[:], sec["g1"][:], ln1b_c[:])
    nc.vector.tensor_add(B1c[:], B1c[:], sec["be1"][:])
    G2c = const.tile([P, KT6], F32, name="G2c")
    nc.vector.tensor_mul(G2c[:], sec["g2"][:], ln2g_c[:])
    B2c = const.tile([P, KT6], F32, name="B2c")
    nc.vector.tensor_mul(B2c[:], sec["g2"][:], ln2b_c[:])
    nc.vector.tensor_add(B2c[:], B2c[:], sec["be2"][:])
    A1c, A2c = sec["a1"], sec["a2"]

    if STAGE < 2:
        nc.sync.dma_start(io["out"][0:P, 0:KT6], G1c[:])
        sc32 = wrk.tile([P, 512], F32, name="w512")[:, 0:SS // P]
        nc.vector.tensor_copy(sc32, silu_cols[:])
        nc.sync.dma_start(io["out"][P:P + P, 0:SS // P], sc32)
        sl32 = wrk.tile([P, 512], F32, name="w512")[0:2, :]
        nc.vector.tensor_copy(sl32, silu_sl[:, 0:512])
        nc.sync.dma_start(io["out"][2 * P:2 * P + 2, 0:512], sl32)
        sl32b = wrk.tile([P, 512], F32, name="w512")[0:2, 0:64]
        nc.vector.tensor_copy(sl32b, silu_sl[:, 512:576])
        nc.sync.dma_start(io["out"][2 * P + 2:2 * P + 4, 0:64], sl32b)
        nc.sync.dma_start(io["out"][2 * P + 4:2 * P + 5, 0:512], temb_sl[:, 0:512])
        nc.sync.dma_start(io["out"][2 * P + 5:2 * P + 6, 0:512], temb_sl[:, 512:1024])
        nc.sync.dma_start(io["out"][2 * P + 6:2 * P + 7, 0:128], temb_sl[:, 1024:1152])
        nc.sync.dma_start(io["out"][300:309, 0:512],
                          temb_flat.rearrange("(a z) -> a z", z=512))
        return

    # ---------- stage 2: LN1 (own 512 tokens, transposed layout) ----------
    xp_cm = tc.tile_pool(name="xp", bufs=1)
    xp = xp_cm.__enter__()
    early_cm = tc.tile_pool(name="early", bufs=1)
    early = early_cm.__enter__()
    hT = early.tile([P, KT6, TOK], BF, name="hT")
    xT_sb = early.tile([P, KT6, TOK], BF, name="xT")
    nc.sync.dma_start(xT_sb[:], io["xT"].rearrange("(k p) t -> p k t", p=P))
    ps_mu = psum_big.tile([P, 512], F32, name="pbig")
    ps_sq = psum_big.tile([P, 512], F32, name="pbig")
    for k in range(KT6):
        xsq = wrk.tile([P, 512], BF, name="xsqa", bufs=2)
        nc.vector.tensor_mul(xsq[:], xT_sb[:, k, :], xT_sb[:, k, :])
        nc.tensor.matmul(ps_mu[:], ones_bf[:], xT_sb[:, k, :],
                         start=(k == 0), stop=(k == KT6 - 1))
        nc.tensor.matmul(ps_sq[:], ones_bf[:], xsq[:],
                         start=(k == 0), stop=(k == KT6 - 1))
    c1t = early.tile([P, TOK], F32, name="c1t")
    c0t = early.tile([P, TOK], F32, name="c0t")
    mu = wrk.tile([P, 512], F32, name="w512")
    nc.vector.tensor_scalar(mu[:], ps_mu[:], 1.0 / H, None, ALU.mult)
    musq = wrk.tile([P, 512], F32, name="w512")
    nc.vector.tensor_mul(musq[:], mu[:], mu[:])
    varme = wrk.tile([P, 512], F32, name="w512")
    nc.vector.scalar_tensor_tensor(varme[:], ps_sq[:], 1.0 / H, musq[:],
                                   ALU.mult, ALU.subtract)
    std = wrk.tile([P, 512], F32, name="w512")
    nc.scalar.activation(std[:], varme[:], AF.Sqrt, bias=eps_ap[:])
    nc.vector.reciprocal(c1t[:], std[:])
    nc.vector.tensor_mul(c0t[:], mu[:], c1t[:])
    # apply: h = (x*c1 - c0) * G1[k] + B1[k]
    for k in range(KT6):
        xn = wrk.tile([P, 512], F32, name="w512")
        nc.vector.tensor_mul(xn[:], xT_sb[:, k, :], c1t[:])
        nc.vector.tensor_sub(xn[:], xn[:], c0t[:])
        nc.vector.tensor_scalar(hT[:, k, :], xn[:],
                                G1c[:, k:k + 1], B1c[:, k:k + 1],
                                ALU.mult, ALU.add)

    if STAGE < 3:
        for k in range(KT6):
            d32 = wrk.tile([P, 512], F32, name="w512")
            nc.vector.tensor_copy(d32[:], hT[:, k, :])
            nc.sync.dma_start(io["out"][P * k:P * (k + 1), :], d32[:])
        early_cm.__exit__(None, None, None)
        xp_cm.__exit__(None, None, None)
        return

    # ---------- stage 3: QKV (own tokens) + K/V AllGather ----------
    qkv_cm = tc.tile_pool(name="qkvw", bufs=1, side="right")
    qkvw = qkv_cm.__enter__()
    Wqkv = qkvw.tile([P, KT6, 3 * H], BF, name="Wqkv")
    for k in range(KT6):
        nc.sync.dma_start(Wqkv[:, k, :], io["wqkv"][P * k:P * (k + 1), :])

    QTs = xp.tile([P, KT6, TOK], BF, name="QTs")
    k_d = dram.tile([H, TOK], BF)
    v_d = dram.tile([TOK, H], BF)
    for m in range(KT6):   # Q^T and K^T (feature-major)
        ps = psum_big.tile([P, 512], F32, name="pbig")
        for k in range(KT6):
            nc.tensor.matmul(ps[:], Wqkv[:, k, P * m:P * (m + 1)],
                             hT[:, k, :], start=(k == 0), stop=(k == KT6 - 1))
        nc.vector.tensor_copy(QTs[:, m, :], ps[:])
        ps = psum_big.tile([P, 512], F32, name="pbig")
        for k in range(KT6):
            nc.tensor.matmul(ps[:], Wqkv[:, k, H + P * m:H + P * (m + 1)],
                             hT[:, k, :], start=(k == 0), stop=(k == KT6 - 1))
        kst = wrk.tile([P, 512], BF, name="kst", bufs=2)
        nc.vector.tensor_copy(kst[:], ps[:])
        nc.sync.dma_start(k_d[P * m:P * (m + 1), :], kst[:])
    for mt in range(MT4):  # V (normal layout)
        for n0, nsz in [(0, 512), (512, 256)]:
            ps = psum_big.tile([P, 512], F32, name="pbig")[:, 0:nsz]
            for k in range(KT6):
                nc.tensor.matmul(ps, hT[:, k, P * mt:P * (mt + 1)],
                                 Wqkv[:, k, 2 * H + n0:2 * H + n0 + nsz],
                                 start=(k == 0), stop=(k == KT6 - 1))
            vst = wrk.tile([P, 512], BF, name="vst", bufs=2)[:, 0:nsz]
            nc.vector.tensor_copy(vst, ps)
            nc.sync.dma_start(v_d[P * mt:P * (mt + 1), n0:n0 + nsz], vst)
    qkv_cm.__exit__(None, None, None)
    early_cm.__exit__(None, None, None)

    k_gath = dram.tile([4, H, TOK], BF)
    nc.gpsimd.collective_compute(
        "AllGather", ALU.bypass, replica_groups=GROUPS4,
        ins=[k_d.opt()], outs=[k_gath.opt()],
    )
    v_gath = dram.tile([4, TOK, H], BF)
    nc.gpsimd.collective_compute(
        "AllGather", ALU.bypass, replica_groups=GROUPS4,
        ins=[v_d.opt()], outs=[v_gath.opt()],
    )

    att_cm = tc.tile_pool(name="attp", bufs=1)
    attp = att_cm.__enter__()
    KTs = attp.tile([P, KT6, NQ4, TOK], BF, name="KTs")
    V_aug = attp.tile([P, TT16, NH, HD + 1], BF, name="Vaug")
    nc.vector.memset(V_aug[:, :, :, HD:HD + 1], 1.0)
    for s in range(4):
        nc.sync.dma_start(KTs[:, :, s, :],
                          k_gath[s:s + 1].rearrange("o (k p) t -> p (o k) t", p=P))
        for mt in range(MT4):
            nc.sync.dma_start(
                V_aug[:, MT4 * s + mt, :, 0:HD],
                v_gath[s:s + 1, P * mt:P * (mt + 1), :]
                .rearrange("o p (h d) -> p (o h) d", d=HD))
    KTs = KTs.rearrange("p k s t -> p k (s t)")

    if STAGE < 4:
        for k in range(KT6):
            d32 = wrk.tile([P, 512], F32, name="w512")
            nc.vector.tensor_copy(d32[:], QTs[:, k, :])
            nc.sync.dma_start(io["out"][P * k:P * (k + 1), :], d32[:])
        att_cm.__exit__(None, None, None)
        xp_cm.__exit__(None, None, None)
        return

    # ---------- stage 4: attention (12 heads, own 512 queries) ----------
    # o_raw[:, h, :] holds [65, 512] (64 feats + sum row 64)
    o_raw = attp.tile([HD + 1, NH, 512], BF, name="o_raw")
    for h in range(NH):
        kf = h // 2
        off = HD * (h % 2)
        ps_o = psum_o.tile([HD + 1, 512], F32, name="po")
        for kt in range(TT16):
            ps_s = psum_big.tile([P, 512], F32, name="pbig")
            nc.tensor.matmul(ps_s[:],
                             KTs[off:off + HD, kf, P * kt:P * (kt + 1)],
                             QTs[off:off + HD, kf, :],
                             start=True, stop=True)
            e_t = eP.tile([P, 512], BF, name="e")
            nc.scalar.activation(e_t[:], ps_s[:], AF.Exp, scale=SCALE)
            nc.tensor.matmul(ps_o[:], V_aug[:, kt, h, :], e_t[:],
                             start=(kt == 0), stop=(kt == TT16 - 1))
        nc.vector.tensor_copy(o_raw[:, h, :], ps_o[:])
    # batched softmax normalization (sum rows live on partition 64)
    sums = small.tile([NH, 512], BF, name="sums", bufs=1)
    nc.sync.dma_start(sums[:], o_raw[HD:HD + 1, :, :])
    recb = small.tile([NH, 512], BF, name="recb", bufs=1)
    with nc.allow_low_precision(reason="softmax norm factor in bf16"):
        nc.vector.reciprocal(recb[:], sums[:])
    rec_d = dram.tile([NH, 512], BF)
    nc.sync.dma_start(rec_d[:], recb[:])
    rec_bc = attp.tile([HD, NH, 512], BF, name="rec_bc")
    nc.sync.dma_start(rec_bc[:], _rep(rec_d[:], HD))
    oT = xp.tile([P, KT6, TOK], BF, name="oT")
    for h in range(NH):
        kf = h // 2
        off = HD * (h % 2)
        if off == 0:
            nc.vector.tensor_mul(oT[0:HD, kf, :], o_raw[0:HD, h, :],
                                 rec_bc[:, h, :])
        else:
            o_n = wrk.tile([P, 512], BF, name="ost", bufs=2)[0:HD, :]
            nc.vector.tensor_mul(o_n, o_raw[0:HD, h, :], rec_bc[:, h, :])
            nc.sync.dma_start(oT[off:off + HD, kf, :], o_n)
    att_cm.__exit__(None, None, None)

    if STAGE < 5:
        for k in range(KT6):
            d32 = wrk.tile([P, 512], F32, name="w512")
            nc.vector.tensor_copy(d32[:], oT[:, k, :])
            nc.sync.dma_start(io["out"][P * k:P * (k + 1), :], d32[:])
        xp_cm.__exit__(None, None, None)
        return

    # ---------- stage 5: mffn (own 512 tokens) ----------
    gT_cm = tc.tile_pool(name="gTp", bufs=1)
    gTp = gT_cm.__enter__()
    xoT = xp.tile([P, KT6, TOK], F32, name="xoT")
    nc.sync.dma_start(xoT[:], io["xoT"].rearrange("(k p) t -> p k t", p=P))

    def mlp1(inT, w1_dram, gT):
        for m in range(FFT):
            ps = psum_big.tile([P, 512], F32, name="pbig")
            for k in range(KT6):
                w_t = wk.tile([P, P], BF, name="wk1")
                nc.sync.dma_start(w_t[:], w1_dram[P * k:P * (k + 1), P * m:P * (m + 1)])
                nc.tensor.matmul(ps[:], w_t[:], inT[:, k, :],
                                 start=(k == 0), stop=(k == KT6 - 1))
            nc.scalar.activation(gT[:, m, :], ps[:], AF.Tanh if SIM_SAFE else AF.Gelu)

    def mlp2_T(gT, w2_dram, ac, res_T, out_T):
        # out_T[:, f, :] = res_T[:, f, :] + ac[f] * (w2.T @ g)  (transposed out)
        for f in range(KT6):
            ps = psum_big.tile([P, 512], F32, name="pbig")
            for k in range(FFT):
                w_t = wk.tile([P, P], BF, name="wk1")
                nc.sync.dma_start(w_t[:], w2_dram[P * k:P * (k + 1), P * f:P * (f + 1)])
                nc.tensor.matmul(ps[:], w_t[:], gT[:, k, :],
                                 start=(k == 0), stop=(k == FFT - 1))
            nc.vector.scalar_tensor_tensor(out_T[:, f, :], ps[:], ac[:, f:f + 1],
                                           res_T[:, f, :], ALU.mult, ALU.add)

    gT = gTp.tile([P, FFT, TOK], BF, name="gT")
    mlp1(oT, io["wm1"], gT)
    x1T = xp.tile([P, KT6, TOK], F32, name="x1T")
    mlp2_T(gT, io["wm2"], A1c, xoT, x1T)

    if STAGE < 6:
        for k in range(KT6):
            nc.sync.dma_start(io["out"][P * k:P * (k + 1), :], x1T[:, k, :])
        gT_cm.__exit__(None, None, None)
        xp_cm.__exit__(None, None, None)
        return

    # ---------- stage 6: LN2 (transposed) ----------
    x1b = xp.tile([P, KT6, TOK], BF, name="x1b")
    ps_mu = psum_big.tile([P, 512], F32, name="pbig")
    ps_sq = psum_big.tile([P, 512], F32, name="pbig")
    for k in range(KT6):
        nc.vector.tensor_copy(x1b[:, k, :], x1T[:, k, :])
        xsq = wrk.tile([P, 512], BF, name="xsqb", bufs=2)
        nc.vector.tensor_mul(xsq[:], x1b[:, k, :], x1b[:, k, :])
        nc.tensor.matmul(ps_mu[:], ones_bf[:], x1b[:, k, :],
                         start=(k == 0), stop=(k == KT6 - 1))
        nc.tensor.matmul(ps_sq[:], ones_bf[:], xsq[:],
                         start=(k == 0), stop=(k == KT6 - 1))
    mu = wrk.tile([P, 512], F32, name="w512")
    nc.vector.tensor_scalar(mu[:], ps_mu[:], 1.0 / H, None, ALU.mult)
    musq = wrk.tile([P, 512], F32, name="w512")
    nc.vector.tensor_mul(musq[:], mu[:], mu[:])
    varme = wrk.tile([P, 512], F32, name="w512")
    nc.vector.scalar_tensor_tensor(varme[:], ps_sq[:], 1.0 / H, musq[:],
                                   ALU.mult, ALU.subtract)
    std = wrk.tile([P, 512], F32, name="w512")
    nc.scalar.activation(std[:], varme[:], AF.Sqrt, bias=eps_ap[:])
    c1 = wrk.tile([P, 512], F32, name="c1ln2")
    nc.vector.reciprocal(c1[:], std[:])
    c0 = wrk.tile([P, 512], F32, name="c0ln2")
    nc.vector.tensor_mul(c0[:], mu[:], c1[:])
    h2T = xp.tile([P, KT6, TOK], BF, name="h2T")
    for k in range(KT6):
        xn = wrk.tile([P, 512], F32, name="w512")
        nc.vector.tensor_mul(xn[:], x1T[:, k, :], c1[:])
        nc.vector.tensor_sub(xn[:], xn[:], c0[:])
        nc.vector.tensor_scalar(h2T[:, k, :], xn[:],
                                G2c[:, k:k + 1], B2c[:, k:k + 1],
                                ALU.mult, ALU.add)

    if STAGE < 7:
        for k in range(KT6):
            nc.sync.dma_start(io["out"][P * k:P * (k + 1), :], x1T[:, k, :])
        gT_cm.__exit__(None, None, None)
        xp_cm.__exit__(None, None, None)
        return

    # ---------- stage 7: FFN ----------
    gT2 = gTp.tile([P, FFT, TOK], BF, name="gT")
    mlp1(h2T, io["wf1"], gT2)
    outT = xp.tile([P, KT6, TOK], F32, name="outT")
    mlp2_T(gT2, io["wf2"], A2c, x1T, outT)
    for k in range(KT6):
        nc.sync.dma_start(io["out"][P * k:P * (k + 1), :], outT[:, k, :])

    gT_cm.__exit__(None, None, None)
    xp_cm.__exit__(None, None, None)


_CACHE = {}


def _build():
    key = (STAGE, SIM_SAFE)
    if key in _CACHE:
        return _CACHE[key]
    nc = bacc.Bacc("TRN2", target_bir_lowering=False, debug=False, num_devices=N_CORES)
    io = {}
    def inp(name, shape, dt):
        io[name] = nc.dram_tensor(name, shape, dt, kind="ExternalInput").ap()
    inp("xT", [H, TOK], BF)
    inp("xoT", [H, TOK], F32)
    inp("t2T", [H, 2], BF)
    inp("wqkv", [H, 3 * H], BF)
    inp("wm1", [H, FF], BF)
    inp("wm2", [FF, H], BF)
    inp("wf1", [H, FF], BF)
    inp("wf2", [FF, H], BF)
    inp("ss1s", [H, SSC], BF)
    inp("ss2s", [SS, SSH], BF)
    inp("ln1g_c", [P, KT6], F32)
    inp("ln1b_c", [P, KT6], F32)
    inp("ln2g_c", [P, KT6], F32)
    inp("ln2b_c", [P, KT6], F32)
    io["out"] = nc.dram_tensor("out", [H, TOK], F32, kind="ExternalOutput").ap()
    from contextlib import ExitStack
    with tile.TileContext(nc) as tc, ExitStack() as ctx:
        _emit(ctx, tc, io)
    nc.compile()
    _CACHE[key] = nc
    return nc


def _bf16(a):
    return np.ascontiguousarray(a.astype(ml_dtypes.bfloat16))


def _cols(v):
    return np.ascontiguousarray(np.asarray(v, np.float32).reshape(KT6, P).T)


def make_in_maps(inputs):
    x = np.asarray(inputs["x"], np.float32)
    t = np.asarray(inputs["t"], np.float32)
    for zname in ("b_qkv", "b_mffn1", "b_mffn2", "b_ss1", "b_ss2", "b_ffn1", "b_ffn2"):
        if np.any(np.asarray(inputs[zname])):
            raise NotImplementedError(f"{zname} must be zero (kernel folds biases away)")

    wqkv = _bf16(inputs["w_qkv"])
    wm1 = _bf16(inputs["w_mffn1"])
    wm2 = _bf16(inputs["w_mffn2"])
    wf1 = _bf16(inputs["w_ffn1"])
    wf2 = _bf16(inputs["w_ffn2"])
    ss1 = np.asarray(inputs["w_ss1"], np.float32)
    ss2 = np.asarray(inputs["w_ss2"], np.float32)
    t2T = _bf16(t.reshape(2, H).T)
    ln = {f"ln{i}{gb}_c": _cols(inputs[f"ln{i}_{gb}"])
          for i in (1, 2) for gb in ("g", "b")}

    in_maps = []
    for c in range(N_CORES):
        b, j = divmod(c, 4)
        xo = x[b, TOK * j:TOK * (j + 1)]
        in_maps.append({
            "xT": _bf16(xo.T),
            "xoT": np.ascontiguousarray(xo.T),
            "t2T": t2T,
            "wqkv": wqkv,
            "wm1": wm1, "wm2": wm2, "wf1": wf1, "wf2": wf2,
            "ss1s": _bf16(ss1[:, SSC * c:SSC * (c + 1)]),
            "ss2s": _bf16(ss2[:, SSH * j:SSH * (j + 1)]),
            **ln,
        })
    return in_maps


def kernel(**inputs):
    in_maps = make_in_maps(inputs)
    nc = _build()
    res = run_bass_kernel_spmd(nc, in_maps, core_ids=list(range(N_CORES)))
    out = np.empty((B, T, H), np.float32)
    for c in range(N_CORES):
        b, j = divmod(c, 4)
        out[b, TOK * j:TOK * (j + 1)] = res.results[c]["out"].T
    return out


# revision 17
# speedup vs baseline: 1.3860x; 1.3860x over previous
"""DiT block kernel for 8 Trainium2 NeuronCores (Bass/Tile, SPMD).

Sharding: core c = 4*b + j handles batch b, tokens 512j..512j+512.
Each core computes LN1 + Q/K/V only for its own 512 tokens; K^T and V
are exchanged with two AllGathers inside the 4-core group, then each
core runs attention for its own queries against all 2048 keys (all 12
heads). The adaLN (scale_shift) first GEMM is column-sharded 8-ways
(one tiny AllToAll also routes each core its batch's row), the second
4-ways per group (one tiny AllGather).

Everything runs in the transposed layout [feature-partitions, tokens]:
per-feature modulation constants are per-partition scalars, per-token
LN stats are matmul-broadcast rows. The second GEMM of each MLP swaps
stationary/moving so its output is already transposed; the final output
is written feature-major and transposed on the host. Softmax
normalization is batched: one [12,512] reciprocal + one broadcast DMA.

All matmuls bf16 with fp32 PSUM accumulation; residual stream is fp32.
"""
import sys
sys.path.insert(0, "/opt/trn_rl_repo")

import numpy as np
import ml_dtypes

import concourse.bass as bass
import concourse.tile as tile
from concourse import bacc, mybir
from concourse.bass_utils import run_bass_kernel_spmd

P = 128
H = 768
NH = 12
HD = 64
B = 2
T = 2048
TOK = 512            # own tokens per core
KT6 = H // P         # 6 k-tiles over hidden
TT16 = T // P        # 16 token tiles over full batch
MT4 = TOK // P       # 4 token tiles over own tokens
NQ4 = T // TOK       # 4 token-quarter shards in a group
FF = 3072
FFT = FF // P        # 24
SS = 6 * H           # 4608
SSC = SS // 8        # 576  ss1 columns per core (8-way)
SSH = SS // 4        # 1152 ss2 columns per core (4-way in group)
SCALE = float(1.0 / np.sqrt(H))
EPS = 1e-5

BF = mybir.dt.bfloat16
F32 = mybir.dt.float32
AF = mybir.ActivationFunctionType
ALU = mybir.AluOpType

N_CORES = 8
STAGE = 9  # emit stages up to this number
SIM_SAFE = False
GROUPS4 = [[0, 1, 2, 3], [4, 5, 6, 7]]
GROUPS8 = [[0, 1, 2, 3, 4, 5, 6, 7]]


def _rep(dram_ap, times):
    """DRAM AP -> AP with a leading 0-stride axis reading it `times` times."""
    return bass.AP(tensor=dram_ap.tensor, offset=dram_ap.offset,
                   ap=[[0, times]] + [list(a) for a in dram_ap.ap])


def _emit(ctx, tc, io):
    nc = tc.nc

    const = ctx.enter_context(tc.tile_pool(name="const", bufs=1))
    psum_big = ctx.enter_context(tc.tile_pool(name="psum_big", bufs=4, space="PSUM"))
    psum_o = ctx.enter_context(tc.tile_pool(name="psum_o", bufs=3, space="PSUM"))
    dram = ctx.enter_context(tc.tile_pool(name="dram", bufs=12, space="DRAM"))
    wrk = ctx.enter_context(tc.tile_pool(name="wrk", bufs=6))
    small = ctx.enter_context(tc.tile_pool(name="small", bufs=8))
    wn = ctx.enter_context(tc.tile_pool(name="wn", bufs=6))
    eP = ctx.enter_context(tc.tile_pool(name="eP", bufs=4))

    # ---------- constants ----------
    ones_bf = const.tile([P, P], BF, name="ones_bf")
    nc.vector.memset(ones_bf[:], 1.0)
    eps_ap = const.tile([P, 1], F32, name="eps")
    nc.vector.memset(eps_ap[:], EPS)

    ln1g_c = const.tile([P, KT6], F32, name="ln1g")
    nc.sync.dma_start(ln1g_c[:], io["ln1g_c"][:])
    ln1b_c = const.tile([P, KT6], F32, name="ln1b")
    nc.sync.dma_start(ln1b_c[:], io["ln1b_c"][:])
    ln2g_c = const.tile([P, KT6], F32, name="ln2g")
    nc.sync.dma_start(ln2g_c[:], io["ln2g_c"][:])
    ln2b_c = const.tile([P, KT6], F32, name="ln2b")
    nc.sync.dma_start(ln2b_c[:], io["ln2b_c"][:])

    # ---------- stage 1: scale_shift (adaLN) ----------
    # ss1, full width locally: [1, 4608] = t_my.T @ ss1 (collective-free)
    tT_sb = const.tile([P, KT6], BF, name="tT")
    nc.sync.dma_start(tT_sb[:], io["tT"].rearrange("(k p) o -> p (k o)", p=P))
    silu_row = small.tile([1, SS], BF, name="silu_row", bufs=1)
    for grp in range(3):           # 3 x 1536 columns
        g0 = 1536 * grp
        pss = [psum_big.tile([P, 512], F32, name="pbig")[0:1, :]
               for _ in range(3)]
        for k in range(KT6):
            w_t = wn.tile([P, 1536], BF, name="wnss", bufs=3)
            nc.sync.dma_start(w_t[:], io["ss1"][P * k:P * (k + 1), g0:g0 + 1536])
            for ch in range(3):
                nc.tensor.matmul(pss[ch], tT_sb[:, k:k + 1],
                                 w_t[:, 512 * ch:512 * (ch + 1)],
                                 start=(k == 0), stop=(k == KT6 - 1))
        for ch in range(3):
            sig = wrk.tile([P, 512], F32, name="w512")[0:1, :]
            nc.scalar.activation(sig, pss[ch], AF.Sigmoid)
            nc.vector.tensor_mul(silu_row[:, g0 + 512 * ch:g0 + 512 * (ch + 1)],
                                 pss[ch], sig)
    # cross-partition: row -> column layout via a DRAM bounce
    silu_dram = dram.tile([1, SS], BF)
    nc.sync.dma_start(silu_dram[:], silu_row[:])
    silu_cols = const.tile([P, SS // P], BF, name="silu_cols")
    nc.sync.dma_start(silu_cols[:], silu_dram.rearrange("o (k p) -> (o p) k", p=P))

    # ss2 (4-way col shard in group): [1, 1152] = silu_my @ ss2_slice
    temb_sl = small.tile([1, SSH], F32, name="temb_sl", bufs=1)
    pss = [psum_big.tile([P, 512], F32, name="pbig")[0:1, 0:nsz]
           for n0, nsz in [(0, 512), (512, 512), (1024, 128)]]
    for k in range(SS // P):    # 36
        w_t = wn.tile([P, SSH], BF, name="wnss", bufs=3)
        nc.sync.dma_start(w_t[:], io["ss2s"][P * k:P * (k + 1), :])
        for ch, (n0, nsz) in enumerate([(0, 512), (512, 512), (1024, 128)]):
            nc.tensor.matmul(pss[ch], silu_cols[:, k:k + 1],
                             w_t[:, n0:n0 + nsz],
                             start=(k == 0), stop=(k == SS // P - 1))
    for ch, (n0, nsz) in enumerate([(0, 512), (512, 512), (1024, 128)]):
        nc.vector.tensor_copy(temb_sl[:, n0:n0 + nsz], pss[ch])

    temb_sm = dram.tile([1, SSH], F32)
    nc.sync.dma_start(temb_sm[:], temb_sl[:])
    temb_out = dram.tile([4, SSH], F32)
    nc.gpsimd.collective_compute(
        "AllGather", ALU.bypass, replica_groups=GROUPS4,
        ins=[temb_sm.opt()], outs=[temb_out.opt()],
    )
    temb_flat = temb_out.rearrange("r g -> (r g)")  # [4608] my batch
    # six sections -> column layout [128, 6] each
    sec = {}
    for i, name in enumerate(["g1", "be1", "a1", "g2", "be2", "a2"]):
        t = const.tile([P, KT6], F32, name=f"sec_{name}")
        nc.sync.dma_start(t[:], temb_flat[H * i:H * (i + 1)]
                          .rearrange("(k p) -> p k", p=P))
        sec[name] = t
    G1c = const.tile([P, KT6], F32, name="G1c")
    nc.vector.tensor_mul(G1c[:], sec["g1"][:], ln1g_c[:])
    B1c = const.tile([P, KT6], F32, name="B1c")
    nc.vector.tensor_mul(B1c[:], sec["g1"][:], ln1b_c[:])
    nc.vector.tensor_add(B1c[:], B1c[:], sec["be1"][:])
    G2c = const.tile([P, KT6], F32, name="G2c")
    nc.vector.tensor_mul(G2c[:], sec["g2"][:], ln2g_c[:])
    B2c = const.tile([P, KT6], F32, name="B2c")
    nc.vector.tensor_mul(B2c[:], sec["g2"][:], ln2b_c[:])
    nc.vector.tensor_add(B2c[:], B2c[:], sec["be2"][:])
    A1c, A2c = sec["a1"], sec["a2"]

    if STAGE < 2:
        nc.sync.dma_start(io["out"][0:P, 0:KT6], G1c[:])
        sc32 = wrk.tile([P, 512], F32, name="w512")[:, 0:SS // P]
        nc.vector.tensor_copy(sc32, silu_cols[:])
        nc.sync.dma_start(io["out"][P:P + P, 0:SS // P], sc32)

        nc.sync.dma_start(io["out"][2 * P + 4:2 * P + 5, 0:512], temb_sl[:, 0:512])
        nc.sync.dma_start(io["out"][2 * P + 5:2 * P + 6, 0:512], temb_sl[:, 512:1024])
        nc.sync.dma_start(io["out"][2 * P + 6:2 * P + 7, 0:128], temb_sl[:, 1024:1152])
        nc.sync.dma_start(io["out"][300:309, 0:512],
                          temb_flat.rearrange("(a z) -> a z", z=512))
        return

    # ---------- stage 2: LN1 (own 512 tokens, transposed layout) ----------
    xp_cm = tc.tile_pool(name="xp", bufs=1)
    xp = xp_cm.__enter__()
    early_cm = tc.tile_pool(name="early", bufs=1)
    early = early_cm.__enter__()
    hT = early.tile([P, KT6, TOK], BF, name="hT")
    xT_sb = early.tile([P, KT6, TOK], BF, name="xT")
    nc.sync.dma_start(xT_sb[:], io["xT"].rearrange("(k p) t -> p k t", p=P))
    ps_mu = psum_big.tile([P, 512], F32, name="pbig")
    ps_sq = psum_big.tile([P, 512], F32, name="pbig")
    for k in range(KT6):
        xsq = wrk.tile([P, 512], BF, name="xsqa", bufs=2)
        nc.vector.tensor_mul(xsq[:], xT_sb[:, k, :], xT_sb[:, k, :])
        nc.tensor.matmul(ps_mu[:], ones_bf[:], xT_sb[:, k, :],
                         start=(k == 0), stop=(k == KT6 - 1))
        nc.tensor.matmul(ps_sq[:], ones_bf[:], xsq[:],
                         start=(k == 0), stop=(k == KT6 - 1))
    c1t = early.tile([P, TOK], F32, name="c1t")
    c0t = early.tile([P, TOK], F32, name="c0t")
    mu = wrk.tile([P, 512], F32, name="w512")
    nc.vector.tensor_scalar(mu[:], ps_mu[:], 1.0 / H, None, ALU.mult)
    musq = wrk.tile([P, 512], F32, name="w512")
    nc.vector.tensor_mul(musq[:], mu[:], mu[:])
    varme = wrk.tile([P, 512], F32, name="w512")
    nc.vector.scalar_tensor_tensor(varme[:], ps_sq[:], 1.0 / H, musq[:],
                                   ALU.mult, ALU.subtract)
    std = wrk.tile([P, 512], F32, name="w512")
    nc.scalar.activation(std[:], varme[:], AF.Sqrt, bias=eps_ap[:])
    nc.vector.reciprocal(c1t[:], std[:])
    nc.vector.tensor_mul(c0t[:], mu[:], c1t[:])
    # apply: h = (x*c1 - c0) * G1[k] + B1[k]
    for k in range(KT6):
        xn = wrk.tile([P, 512], F32, name="w512")
        nc.vector.tensor_mul(xn[:], xT_sb[:, k, :], c1t[:])
        nc.vector.tensor_sub(xn[:], xn[:], c0t[:])
        nc.vector.tensor_scalar(hT[:, k, :], xn[:],
                                G1c[:, k:k + 1], B1c[:, k:k + 1],
                                ALU.mult, ALU.add)

    if STAGE < 3:
        for k in range(KT6):
            d32 = wrk.tile([P, 512], F32, name="w512")
            nc.vector.tensor_copy(d32[:], hT[:, k, :])
            nc.sync.dma_start(io["out"][P * k:P * (k + 1), :], d32[:])
        early_cm.__exit__(None, None, None)
        xp_cm.__exit__(None, None, None)
        return

    # ---------- stage 3: QKV (own tokens) + K/V AllGather ----------
    qkv_cm = tc.tile_pool(name="qkvw", bufs=1, side="right")
    qkvw = qkv_cm.__enter__()
    Wqkv = qkvw.tile([P, KT6, 3 * H], BF, name="Wqkv")
    for k in range(KT6):
        nc.sync.dma_start(Wqkv[:, k, :], io["wqkv"][P * k:P * (k + 1), :])

    QTs = xp.tile([P, KT6, TOK], BF, name="QTs")
    k_d = dram.tile([H, TOK], BF)
    v_d = dram.tile([TOK, H], BF)
    for m in range(KT6):   # Q^T and K^T (feature-major)
        ps = psum_big.tile([P, 512], F32, name="pbig")
        for k in range(KT6):
            nc.tensor.matmul(ps[:], Wqkv[:, k, P * m:P * (m + 1)],
                             hT[:, k, :], start=(k == 0), stop=(k == KT6 - 1))
        nc.vector.tensor_copy(QTs[:, m, :], ps[:])
        ps = psum_big.tile([P, 512], F32, name="pbig")
        for k in range(KT6):
            nc.tensor.matmul(ps[:], Wqkv[:, k, H + P * m:H + P * (m + 1)],
                             hT[:, k, :], start=(k == 0), stop=(k == KT6 - 1))
        kst = wrk.tile([P, 512], BF, name="kst", bufs=2)
        nc.vector.tensor_copy(kst[:], ps[:])
        nc.sync.dma_start(k_d[P * m:P * (m + 1), :], kst[:])
    for mt in range(MT4):  # V (normal layout)
        for n0, nsz in [(0, 512), (512, 256)]:
            ps = psum_big.tile([P, 512], F32, name="pbig")[:, 0:nsz]
            for k in range(KT6):
                nc.tensor.matmul(ps, hT[:, k, P * mt:P * (mt + 1)],
                                 Wqkv[:, k, 2 * H + n0:2 * H + n0 + nsz],
                                 start=(k == 0), stop=(k == KT6 - 1))
            vst = wrk.tile([P, 512], BF, name="vst", bufs=2)[:, 0:nsz]
            nc.vector.tensor_copy(vst, ps)
            nc.sync.dma_start(v_d[P * mt:P * (mt + 1), n0:n0 + nsz], vst)
    qkv_cm.__exit__(None, None, None)
    early_cm.__exit__(None, None, None)

    k_gath = dram.tile([4, H, TOK], BF)
    nc.gpsimd.collective_compute(
        "AllGather", ALU.bypass, replica_groups=GROUPS4,
        ins=[k_d.opt()], outs=[k_gath.opt()],
    )
    v_gath = dram.tile([4, TOK, H], BF)
    nc.gpsimd.collective_compute(
        "AllGather", ALU.bypass, replica_groups=GROUPS4,
        ins=[v_d.opt()], outs=[v_gath.opt()],
    )

    att_cm = tc.tile_pool(name="attp", bufs=1)
    attp = att_cm.__enter__()
    KTs = attp.tile([P, KT6, NQ4, TOK], BF, name="KTs")
    V_aug = attp.tile([P, TT16, NH, HD + 1], BF, name="Vaug")
    nc.vector.memset(V_aug[:, :, :, HD:HD + 1], 1.0)
    for s in range(4):
        nc.sync.dma_start(KTs[:, :, s, :],
                          k_gath[s:s + 1].rearrange("o (k p) t -> p (o k) t", p=P))
        for mt in range(MT4):
            nc.sync.dma_start(
                V_aug[:, MT4 * s + mt, :, 0:HD],
                v_gath[s:s + 1, P * mt:P * (mt + 1), :]
                .rearrange("o p (h d) -> p (o h) d", d=HD))
    KTs = KTs.rearrange("p k s t -> p k (s t)")

    if STAGE < 4:
        for k in range(KT6):
            d32 = wrk.tile([P, 512], F32, name="w512")
            nc.vector.tensor_copy(d32[:], QTs[:, k, :])
            nc.sync.dma_start(io["out"][P * k:P * (k + 1), :], d32[:])
        att_cm.__exit__(None, None, None)
        xp_cm.__exit__(None, None, None)
        return

    # ---------- stage 4: attention (12 heads, own 512 queries) ----------
    # o_raw[:, h, :] holds [65, 512] (64 feats + sum row 64)
    o_raw = attp.tile([HD + 1, NH, 512], BF, name="o_raw")
    for hp in range(NH // 2):
        kf = hp
        ps_os = [psum_o.tile([HD + 1, 512], F32, name="po") for _ in range(2)]
        for kt in range(TT16):
            for sub in range(2):
                off = HD * sub
                ps_s = psum_big.tile([P, 512], F32, name="pbig")
                nc.tensor.matmul(ps_s[:],
                                 KTs[off:off + HD, kf, P * kt:P * (kt + 1)],
                                 QTs[off:off + HD, kf, :],
                                 start=True, stop=True)
                e_t = eP.tile([P, 512], BF, name="e")
                nc.scalar.activation(e_t[:], ps_s[:], AF.Exp, scale=SCALE)
                nc.tensor.matmul(ps_os[sub][:], V_aug[:, kt, 2 * hp + sub, :],
                                 e_t[:], start=(kt == 0), stop=(kt == TT16 - 1))
        for sub in range(2):
            nc.vector.tensor_copy(o_raw[:, 2 * hp + sub, :], ps_os[sub][:])
    # batched softmax normalization (sum rows live on partition 64)
    sums = small.tile([NH, 512], BF, name="sums", bufs=1)
    nc.sync.dma_start(sums[:], o_raw[HD:HD + 1, :, :])
    recb = small.tile([NH, 512], BF, name="recb", bufs=1)
    with nc.allow_low_precision(reason="softmax norm factor in bf16"):
        nc.vector.reciprocal(recb[:], sums[:])
    rec_d = dram.tile([NH, 512], BF)
    nc.sync.dma_start(rec_d[:], recb[:])
    rec_bc = attp.tile([HD, NH, 512], BF, name="rec_bc")
    nc.sync.dma_start(rec_bc[:], _rep(rec_d[:], HD))
    oT = xp.tile([P, KT6, TOK], BF, name="oT")
    for h in range(NH):
        kf = h // 2
        off = HD * (h % 2)
        if off == 0:
            nc.vector.tensor_mul(oT[0:HD, kf, :], o_raw[0:HD, h, :],
                                 rec_bc[:, h, :])
        else:
            o_n = wrk.tile([P, 512], BF, name="ost", bufs=2)[0:HD, :]
            nc.vector.tensor_mul(o_n, o_raw[0:HD, h, :], rec_bc[:, h, :])
            nc.sync.dma_start(oT[off:off + HD, kf, :], o_n)
    att_cm.__exit__(None, None, None)

    if STAGE < 5:
        for k in range(KT6):
            d32 = wrk.tile([P, 512], F32, name="w512")
            nc.vector.tensor_copy(d32[:], oT[:, k, :])
            nc.sync.dma_start(io["out"][P * k:P * (k + 1), :], d32[:])
        xp_cm.__exit__(None, None, None)
        return

    # ---------- stage 5: mffn (own 512 tokens) ----------
    gT_cm = tc.tile_pool(name="gTp", bufs=1)
    gTp = gT_cm.__enter__()
    wk_cm = tc.tile_pool(name="wk", bufs=1)
    wk = wk_cm.__enter__()
    xoT = xp.tile([P, KT6, TOK], F32, name="xoT")
    nc.sync.dma_start(xoT[:], io["xoT"].rearrange("(k p) t -> p k t", p=P))

    def mlp1(inT, w1_dram, gT):
        w1sb = wk.tile([P, KT6, FF], BF, name="wmlp", bufs=1)
        for k in range(KT6):
            nc.sync.dma_start(w1sb[:, k, :], w1_dram[P * k:P * (k + 1), :])
        for m in range(FFT):
            ps = psum_big.tile([P, 512], F32, name="pbig")
            for k in range(KT6):
                nc.tensor.matmul(ps[:], w1sb[:, k, P * m:P * (m + 1)],
                                 inT[:, k, :], start=(k == 0), stop=(k == KT6 - 1))
            nc.scalar.activation(gT[:, m, :], ps[:], AF.Tanh if SIM_SAFE else AF.Gelu)

    def mlp2_T(gT, w2_dram, ac, res_T, out_T):
        # out_T[:, f, :] = res_T[:, f, :] + ac[f] * (w2.T @ g)  (transposed out)
        w2sb = wk.tile([P, FFT, H], BF, name="wmlp", bufs=1)
        for k in range(FFT):
            nc.sync.dma_start(w2sb[:, k, :], w2_dram[P * k:P * (k + 1), :])
        for f in range(KT6):
            ps = psum_big.tile([P, 512], F32, name="pbig")
            for k in range(FFT):
                nc.tensor.matmul(ps[:], w2sb[:, k, P * f:P * (f + 1)],
                                 gT[:, k, :], start=(k == 0), stop=(k == FFT - 1))
            nc.vector.scalar_tensor_tensor(out_T[:, f, :], ps[:], ac[:, f:f + 1],
                                           res_T[:, f, :], ALU.mult, ALU.add)

    gT = gTp.tile([P, FFT, TOK], BF, name="gT")
    mlp1(oT, io["wm1"], gT)
    x1T = xp.tile([P, KT6, TOK], F32, name="x1T")
    mlp2_T(gT, io["wm2"], A1c, xoT, x1T)

    if STAGE < 6:
        for k in range(KT6):
            nc.sync.dma_start(io["out"][P * k:P * (k + 1), :], x1T[:, k, :])
        wk_cm.__exit__(None, None, None)
        gT_cm.__exit__(None, None, None)
        xp_cm.__exit__(None, None, None)
        return

    # ---------- stage 6: LN2 (transposed) ----------
    x1b = xp.tile([P, KT6, TOK], BF, name="x1b")
    ps_mu = psum_big.tile([P, 512], F32, name="pbig")
    ps_sq = psum_big.tile([P, 512], F32, name="pbig")
    for k in range(KT6):
        nc.vector.tensor_copy(x1b[:, k, :], x1T[:, k, :])
        xsq = wrk.tile([P, 512], BF, name="xsqb", bufs=2)
        nc.vector.tensor_mul(xsq[:], x1b[:, k, :], x1b[:, k, :])
        nc.tensor.matmul(ps_mu[:], ones_bf[:], x1b[:, k, :],
                         start=(k == 0), stop=(k == KT6 - 1))
        nc.tensor.matmul(ps_sq[:], ones_bf[:], xsq[:],
                         start=(k == 0), stop=(k == KT6 - 1))
    mu = wrk.tile([P, 512], F32, name="w512")
    nc.vector.tensor_scalar(mu[:], ps_mu[:], 1.0 / H, None, ALU.mult)
    musq = wrk.tile([P, 512], F32, name="w512")
    nc.vector.tensor_mul(musq[:], mu[:], mu[:])
    varme = wrk.tile([P, 512], F32, name="w512")
    nc.vector.scalar_tensor_tensor(varme[:], ps_sq[:], 1.0 / H, musq[:],
                                   ALU.mult, ALU.subtract)
    std = wrk.tile([P, 512], F32, name="w512")
    nc.scalar.activation(std[:], varme[:], AF.Sqrt, bias=eps_ap[:])
    c1 = wrk.tile([P, 512], F32, name="c1ln2")
    nc.vector.reciprocal(c1[:], std[:])
    c0 = wrk.tile([P, 512], F32, name="c0ln2")
    nc.vector.tensor_mul(c0[:], mu[:], c1[:])
    h2T = xp.tile([P, KT6, TOK], BF, name="h2T")
    for k in range(KT6):
        xn = wrk.tile([P, 512], F32, name="w512")
        nc.vector.tensor_mul(xn[:], x1T[:, k, :], c1[:])
        nc.vector.tensor_sub(xn[:], xn[:], c0[:])
        nc.vector.tensor_scalar(h2T[:, k, :], xn[:],
                                G2c[:, k:k + 1], B2c[:, k:k + 1],
                                ALU.mult, ALU.add)

    if STAGE < 7:
        for k in range(KT6):
            nc.sync.dma_start(io["out"][P * k:P * (k + 1), :], x1T[:, k, :])
        wk_cm.__exit__(None, None, None)
        gT_cm.__exit__(None, None, None)
        xp_cm.__exit__(None, None, None)
        return

    # ---------- stage 7: FFN ----------
    gT2 = gTp.tile([P, FFT, TOK], BF, name="gT")
    mlp1(h2T, io["wf1"], gT2)
    outT = xp.tile([P, KT6, TOK], F32, name="outT")
    mlp2_T(gT2, io["wf2"], A2c, x1T, outT)
    for k in range(KT6):
        nc.sync.dma_start(io["out"][P * k:P * (k + 1), :], outT[:, k, :])

    wk_cm.__exit__(None, None, None)
    gT_cm.__exit__(None, None, None)
    xp_cm.__exit__(None, None, None)


_CACHE = {}


def _build():
    key = (STAGE, SIM_SAFE)
    if key in _CACHE:
        return _CACHE[key]
    nc = bacc.Bacc("TRN2", target_bir_lowering=False, debug=False, num_devices=N_CORES)
    io = {}
    def inp(name, shape, dt):
        io[name] = nc.dram_tensor(name, shape, dt, kind="ExternalInput").ap()
    inp("xT", [H, TOK], BF)
    inp("xoT", [H, TOK], F32)
    inp("tT", [H, 1], BF)
    inp("wqkv", [H, 3 * H], BF)
    inp("wm1", [H, FF], BF)
    inp("wm2", [FF, H], BF)
    inp("wf1", [H, FF], BF)
    inp("wf2", [FF, H], BF)
    inp("ss1", [H, SS], BF)
    inp("ss2s", [SS, SSH], BF)
    inp("ln1g_c", [P, KT6], F32)
    inp("ln1b_c", [P, KT6], F32)
    inp("ln2g_c", [P, KT6], F32)
    inp("ln2b_c", [P, KT6], F32)
    io["out"] = nc.dram_tensor("out", [H, TOK], F32, kind="ExternalOutput").ap()
    from contextlib import ExitStack
    with tile.TileContext(nc) as tc, ExitStack() as ctx:
        _emit(ctx, tc, io)
    nc.compile()
    _CACHE[key] = nc
    return nc


def _bf16(a):
    return np.ascontiguousarray(a.astype(ml_dtypes.bfloat16))


def _cols(v):
    return np.ascontiguousarray(np.asarray(v, np.float32).reshape(KT6, P).T)


def make_in_maps(inputs):
    x = np.asarray(inputs["x"], np.float32)
    t = np.asarray(inputs["t"], np.float32)
    for zname in ("b_qkv", "b_mffn1", "b_mffn2", "b_ss1", "b_ss2", "b_ffn1", "b_ffn2"):
        if np.any(np.asarray(inputs[zname])):
            raise NotImplementedError(f"{zname} must be zero (kernel folds biases away)")

    wqkv = _bf16(inputs["w_qkv"])
    wm1 = _bf16(inputs["w_mffn1"])
    wm2 = _bf16(inputs["w_mffn2"])
    wf1 = _bf16(inputs["w_ffn1"])
    wf2 = _bf16(inputs["w_ffn2"])
    ss1 = np.asarray(inputs["w_ss1"], np.float32)
    ss2 = np.asarray(inputs["w_ss2"], np.float32)
    t2T = _bf16(t.reshape(2, H).T)
    ss1_bf = _bf16(ss1)
    ln = {f"ln{i}{gb}_c": _cols(inputs[f"ln{i}_{gb}"])
          for i in (1, 2) for gb in ("g", "b")}

    in_maps = []
    for c in range(N_CORES):
        b, j = divmod(c, 4)
        xo = x[b, TOK * j:TOK * (j + 1)]
        in_maps.append({
            "xT": _bf16(xo.T),
            "xoT": np.ascontiguousarray(xo.T),
            "tT": np.ascontiguousarray(t2T[:, b:b + 1]),
            "wqkv": wqkv,
            "wm1": wm1, "wm2": wm2, "wf1": wf1, "wf2": wf2,
            "ss1": ss1_bf,
            "ss2s": _bf16(ss2[:, SSH * j:SSH * (j + 1)]),
            **ln,
        })
    return in_maps


def kernel(**inputs):
    in_maps = make_in_maps(inputs)
    nc = _build()
    res = run_bass_kernel_spmd(nc, in_maps, core_ids=list(range(N_CORES)))
    out = np.empty((B, T, H), np.float32)
    for c in range(N_CORES):
        b, j = divmod(c, 4)
        out[b, TOK * j:TOK * (j + 1)] = res.results[c]["out"].T
    return out


# revision 19
# speedup vs baseline: 1.4754x; 1.0645x over previous
"""DiT block kernel for 8 Trainium2 NeuronCores (Bass/Tile, SPMD).

Sharding: core c = 4*b + j handles batch b, tokens 512j..512j+512.
Each core computes LN1 + Q/K/V only for its own 512 tokens; K^T and V
are exchanged with two AllGathers inside the 4-core group, then each
core runs attention for its own queries against all 2048 keys (all 12
heads). The adaLN (scale_shift) first GEMM is column-sharded 8-ways
(one tiny AllToAll also routes each core its batch's row), the second
4-ways per group (one tiny AllGather).

Everything runs in the transposed layout [feature-partitions, tokens]:
per-feature modulation constants are per-partition scalars, per-token
LN stats are matmul-broadcast rows. The second GEMM of each MLP swaps
stationary/moving so its output is already transposed; the final output
is written feature-major and transposed on the host. Softmax
normalization is batched: one [12,512] reciprocal + one broadcast DMA.

All matmuls bf16 with fp32 PSUM accumulation; residual stream is fp32.
"""
import sys
sys.path.insert(0, "/opt/trn_rl_repo")

import numpy as np
import ml_dtypes

import concourse.bass as bass
import concourse.tile as tile
from concourse import bacc, mybir
from concourse.bass_utils import run_bass_kernel_spmd

P = 128
H = 768
NH = 12
HD = 64
B = 2
T = 2048
TOK = 512            # own tokens per core
KT6 = H // P         # 6 k-tiles over hidden
TT16 = T // P        # 16 token tiles over full batch
MT4 = TOK // P       # 4 token tiles over own tokens
NQ4 = T // TOK       # 4 token-quarter shards in a group
FF = 3072
FFT = FF // P        # 24
SS = 6 * H           # 4608
SSC = SS // 8        # 576  ss1 columns per core (8-way)
SSH = SS // 4        # 1152 ss2 columns per core (4-way in group)
SCALE = float(1.0 / np.sqrt(H))
EPS = 1e-5

BF = mybir.dt.bfloat16
F8 = mybir.dt.float8e4
F32 = mybir.dt.float32
AF = mybir.ActivationFunctionType
ALU = mybir.AluOpType

N_CORES = 8
STAGE = 9  # emit stages up to this number
SIM_SAFE = False
GROUPS4 = [[0, 1, 2, 3], [4, 5, 6, 7]]
GROUPS8 = [[0, 1, 2, 3, 4, 5, 6, 7]]


def _rep(dram_ap, times):
    """DRAM AP -> AP with a leading 0-stride axis reading it `times` times."""
    return bass.AP(tensor=dram_ap.tensor, offset=dram_ap.offset,
                   ap=[[0, times]] + [list(a) for a in dram_ap.ap])


def _emit(ctx, tc, io):
    nc = tc.nc

    const = ctx.enter_context(tc.tile_pool(name="const", bufs=1))
    psum_big = ctx.enter_context(tc.tile_pool(name="psum_big", bufs=5, space="PSUM"))
    psum_o = ctx.enter_context(tc.tile_pool(name="psum_o", bufs=3, space="PSUM"))
    dram = ctx.enter_context(tc.tile_pool(name="dram", bufs=12, space="DRAM"))
    wrk = ctx.enter_context(tc.tile_pool(name="wrk", bufs=6))
    small = ctx.enter_context(tc.tile_pool(name="small", bufs=8))
    wn = ctx.enter_context(tc.tile_pool(name="wn", bufs=6))
    eP = ctx.enter_context(tc.tile_pool(name="eP", bufs=4))

    # ---------- constants ----------
    ones_bf = const.tile([P, P], BF, name="ones_bf")
    nc.vector.memset(ones_bf[:], 1.0)
    eps_ap = const.tile([P, 1], F32, name="eps")
    nc.vector.memset(eps_ap[:], EPS)

    ln1g_c = const.tile([P, KT6], F32, name="ln1g")
    nc.sync.dma_start(ln1g_c[:], io["ln1g_c"][:])
    ln1b_c = const.tile([P, KT6], F32, name="ln1b")
    nc.sync.dma_start(ln1b_c[:], io["ln1b_c"][:])
    ln2g_c = const.tile([P, KT6], F32, name="ln2g")
    nc.sync.dma_start(ln2g_c[:], io["ln2g_c"][:])
    ln2b_c = const.tile([P, KT6], F32, name="ln2b")
    nc.sync.dma_start(ln2b_c[:], io["ln2b_c"][:])

    # ---------- stage 1: scale_shift (adaLN) ----------
    # ss1, full width locally: [1, 4608] = t_my.T @ ss1 (collective-free)
    tT_sb = const.tile([P, KT6], BF, name="tT")
    nc.sync.dma_start(tT_sb[:], io["tT"].rearrange("(k p) o -> p (k o)", p=P))
    silu_row = small.tile([1, SS], F8, name="silu_row", bufs=1)
    for grp in range(3):           # 3 x 1536 columns
        g0 = 1536 * grp
        pss = [psum_big.tile([P, 512], F32, name="pbig")[0:1, :]
               for _ in range(3)]
        for k in range(KT6):
            w_t = wn.tile([P, 1536], BF, name="wnss", bufs=3)
            nc.sync.dma_start(w_t[:], io["ss1"][P * k:P * (k + 1), g0:g0 + 1536])
            for ch in range(3):
                nc.tensor.matmul(pss[ch], tT_sb[:, k:k + 1],
                                 w_t[:, 512 * ch:512 * (ch + 1)],
                                 start=(k == 0), stop=(k == KT6 - 1))
        for ch in range(3):
            sig = wrk.tile([P, 512], F32, name="w512")[0:1, :]
            nc.scalar.activation(sig, pss[ch], AF.Sigmoid)
            nc.vector.tensor_mul(silu_row[:, g0 + 512 * ch:g0 + 512 * (ch + 1)],
                                 pss[ch], sig)
    # cross-partition: row -> column layout via a DRAM bounce
    silu_dram = dram.tile([1, SS], F8)
    nc.sync.dma_start(silu_dram[:], silu_row[:])
    silu_cols = const.tile([P, SS // P], F8, name="silu_cols")
    nc.sync.dma_start(silu_cols[:], silu_dram.rearrange("o (k p) -> (o p) k", p=P))

    # ss2 (4-way col shard in group): [1, 1152] = silu_my @ ss2_slice
    temb_sl = small.tile([1, SSH], F32, name="temb_sl", bufs=1)
    pss = [psum_big.tile([P, 512], F32, name="pbig")[0:1, 0:nsz]
           for n0, nsz in [(0, 512), (512, 512), (1024, 128)]]
    for k in range(SS // P):    # 36
        w_t = wn.tile([P, SSH], F8, name="wnss", bufs=3)
        nc.sync.dma_start(w_t[:], io["ss2s"][P * k:P * (k + 1), :])
        for ch, (n0, nsz) in enumerate([(0, 512), (512, 512), (1024, 128)]):
            nc.tensor.matmul(pss[ch], silu_cols[:, k:k + 1],
                             w_t[:, n0:n0 + nsz],
                             start=(k == 0), stop=(k == SS // P - 1))
    for ch, (n0, nsz) in enumerate([(0, 512), (512, 512), (1024, 128)]):
        nc.vector.tensor_scalar(temb_sl[:, n0:n0 + nsz], pss[ch],
                                1.0 / 64.0, None, ALU.mult)

    temb_sm = dram.tile([1, SSH], F32)
    nc.sync.dma_start(temb_sm[:], temb_sl[:])
    temb_out = dram.tile([4, SSH], F32)
    nc.gpsimd.collective_compute(
        "AllGather", ALU.bypass, replica_groups=GROUPS4,
        ins=[temb_sm.opt()], outs=[temb_out.opt()],
    )
    temb_flat = temb_out.rearrange("r g -> (r g)")  # [4608] my batch
    # six sections -> column layout [128, 6] each
    sec = {}
    for i, name in enumerate(["g1", "be1", "a1", "g2", "be2", "a2"]):
        t = const.tile([P, KT6], F32, name=f"sec_{name}")
        nc.sync.dma_start(t[:], temb_flat[H * i:H * (i + 1)]
                          .rearrange("(k p) -> p k", p=P))
        sec[name] = t
    G1c = const.tile([P, KT6], F32, name="G1c")
    nc.vector.tensor_mul(G1c[:], sec["g1"][:], ln1g_c[:])
    B1c = const.tile([P, KT6], F32, name="B1c")
    nc.vector.tensor_mul(B1c[:], sec["g1"][:], ln1b_c[:])
    nc.vector.tensor_add(B1c[:], B1c[:], sec["be1"][:])
    G2c = const.tile([P, KT6], F32, name="G2c")
    nc.vector.tensor_mul(G2c[:], sec["g2"][:], ln2g_c[:])
    B2c = const.tile([P, KT6], F32, name="B2c")
    nc.vector.tensor_mul(B2c[:], sec["g2"][:], ln2b_c[:])
    nc.vector.tensor_add(B2c[:], B2c[:], sec["be2"][:])
    A1c, A2c = sec["a1"], sec["a2"]

    if STAGE < 2:
        nc.sync.dma_start(io["out"][0:P, 0:KT6], G1c[:])
        sc32 = wrk.tile([P, 512], F32, name="w512")[:, 0:SS // P]
        nc.vector.tensor_copy(sc32, silu_cols[:])
        nc.sync.dma_start(io["out"][P:P + P, 0:SS // P], sc32)

        nc.sync.dma_start(io["out"][2 * P + 4:2 * P + 5, 0:512], temb_sl[:, 0:512])
        nc.sync.dma_start(io["out"][2 * P + 5:2 * P + 6, 0:512], temb_sl[:, 512:1024])
        nc.sync.dma_start(io["out"][2 * P + 6:2 * P + 7, 0:128], temb_sl[:, 1024:1152])
        nc.sync.dma_start(io["out"][300:309, 0:512],
                          temb_flat.rearrange("(a z) -> a z", z=512))
        return

    # ---------- stage 2: LN1 (own 512 tokens, transposed layout) ----------
    xp_cm = tc.tile_pool(name="xp", bufs=1)
    xp = xp_cm.__enter__()
    early_cm = tc.tile_pool(name="early", bufs=1)
    early = early_cm.__enter__()
    hT = early.tile([P, KT6, TOK], BF, name="hT")
    xT_sb = early.tile([P, KT6, TOK], BF, name="xT")
    nc.sync.dma_start(xT_sb[:], io["xT"].rearrange("(k p) t -> p k t", p=P))
    ps_mu = psum_big.tile([P, 512], F32, name="pbig")
    ps_sq = psum_big.tile([P, 512], F32, name="pbig")
    for k in range(KT6):
        xsq = wrk.tile([P, 512], BF, name="xsqa", bufs=2)
        nc.vector.tensor_mul(xsq[:], xT_sb[:, k, :], xT_sb[:, k, :])
        nc.tensor.matmul(ps_mu[:], ones_bf[:], xT_sb[:, k, :],
                         start=(k == 0), stop=(k == KT6 - 1))
        nc.tensor.matmul(ps_sq[:], ones_bf[:], xsq[:],
                         start=(k == 0), stop=(k == KT6 - 1))
    c1t = early.tile([P, TOK], F32, name="c1t")
    c0t = early.tile([P, TOK], F32, name="c0t")
    mu = wrk.tile([P, 512], F32, name="w512")
    nc.vector.tensor_scalar(mu[:], ps_mu[:], 1.0 / H, None, ALU.mult)
    musq = wrk.tile([P, 512], F32, name="w512")
    nc.vector.tensor_mul(musq[:], mu[:], mu[:])
    varme = wrk.tile([P, 512], F32, name="w512")
    nc.vector.scalar_tensor_tensor(varme[:], ps_sq[:], 1.0 / H, musq[:],
                                   ALU.mult, ALU.subtract)
    std = wrk.tile([P, 512], F32, name="w512")
    nc.scalar.activation(std[:], varme[:], AF.Sqrt, bias=eps_ap[:])
    nc.vector.reciprocal(c1t[:], std[:])
    nc.vector.tensor_mul(c0t[:], mu[:], c1t[:])
    # apply: h = (x*c1 - c0) * G1[k] + B1[k]
    for k in range(KT6):
        xn = wrk.tile([P, 512], F32, name="w512")
        nc.vector.tensor_mul(xn[:], xT_sb[:, k, :], c1t[:])
        nc.vector.tensor_sub(xn[:], xn[:], c0t[:])
        nc.vector.tensor_scalar(hT[:, k, :], xn[:],
                                G1c[:, k:k + 1], B1c[:, k:k + 1],
                                ALU.mult, ALU.add)

    if STAGE < 3:
        for k in range(KT6):
            d32 = wrk.tile([P, 512], F32, name="w512")
            nc.vector.tensor_copy(d32[:], hT[:, k, :])
            nc.sync.dma_start(io["out"][P * k:P * (k + 1), :], d32[:])
        early_cm.__exit__(None, None, None)
        xp_cm.__exit__(None, None, None)
        return

    # ---------- stage 3: QKV (own tokens) + K/V AllGather ----------
    qkv_cm = tc.tile_pool(name="qkvw", bufs=1, side="right")
    qkvw = qkv_cm.__enter__()
    Wqkv = qkvw.tile([P, KT6, 3 * H], BF, name="Wqkv")
    for k in range(KT6):
        nc.sync.dma_start(Wqkv[:, k, :], io["wqkv"][P * k:P * (k + 1), :])

    QTs = xp.tile([P, KT6, TOK], F8, name="QTs")
    kv_d = dram.tile([2, H * TOK], F8)
    k_view = kv_d[0:1, :].rearrange("o (r t) -> (o r) t", t=TOK)
    for m in range(KT6):   # K^T (feature-major) first: gather starts earlier
        ps = psum_big.tile([P, 512], F32, name="pbig")
        for k in range(KT6):
            nc.tensor.matmul(ps[:], Wqkv[:, k, H + P * m:H + P * (m + 1)],
                             hT[:, k, :], start=(k == 0), stop=(k == KT6 - 1))
        kst = wrk.tile([P, 512], F8, name="kst", bufs=2)
        nc.vector.tensor_copy(kst[:], ps[:])
        nc.sync.dma_start(k_view[P * m:P * (m + 1), :], kst[:])
    for mt in range(MT4):  # V (normal layout)
        for n0, nsz in [(0, 512), (512, 256)]:
            ps = psum_big.tile([P, 512], F32, name="pbig")[:, 0:nsz]
            for k in range(KT6):
                nc.tensor.matmul(ps, hT[:, k, P * mt:P * (mt + 1)],
                                 Wqkv[:, k, 2 * H + n0:2 * H + n0 + nsz],
                                 start=(k == 0), stop=(k == KT6 - 1))
            vst = wrk.tile([P, 512], F8, name="vst", bufs=2)[:, 0:nsz]
            nc.vector.tensor_copy(vst, ps)
            nc.sync.dma_start(
                kv_d[1:2, :].rearrange("o (p f) -> (o p) f", f=H)
                [P * mt:P * (mt + 1), n0:n0 + nsz], vst)

    kv_g = dram.tile([4, 2, H * TOK], F8)
    nc.gpsimd.collective_compute(
        "AllGather", ALU.bypass, replica_groups=GROUPS4,
        ins=[kv_d.opt()], outs=[kv_g.opt()],
    )

    for m in range(KT6):   # Q^T: overlaps the gather
        ps = psum_big.tile([P, 512], F32, name="pbig")
        for k in range(KT6):
            nc.tensor.matmul(ps[:], Wqkv[:, k, P * m:P * (m + 1)],
                             hT[:, k, :], start=(k == 0), stop=(k == KT6 - 1))
        nc.vector.tensor_copy(QTs[:, m, :], ps[:])
    qkv_cm.__exit__(None, None, None)
    early_cm.__exit__(None, None, None)

    att_cm = tc.tile_pool(name="attp", bufs=1)
    attp = att_cm.__enter__()
    KTs = attp.tile([P, KT6, NQ4, TOK], F8, name="KTs")
    V_aug = attp.tile([P, TT16, NH, HD + 1], F8, name="Vaug")
    nc.vector.memset(V_aug[:, :, :, HD:HD + 1], 1.0)
    for s in range(4):
        nc.sync.dma_start(
            KTs[:, :, s, :],
            kv_g[s:s + 1, 0:1, :].rearrange("a b (k p t) -> p (a b k) t",
                                            p=P, t=TOK))
        for mt in range(MT4):
            nc.sync.dma_start(
                V_aug[:, MT4 * s + mt, :, 0:HD],
                kv_g[s:s + 1, 1:2, 768 * P * mt:768 * P * (mt + 1)]
                .rearrange("a b (p h d) -> p (a b h) d", p=P, d=HD))
    KTs = KTs.rearrange("p k s t -> p k (s t)")

    if STAGE < 4:
        for k in range(KT6):
            d32 = wrk.tile([P, 512], F32, name="w512")
            nc.vector.tensor_copy(d32[:], QTs[:, k, :])
            nc.sync.dma_start(io["out"][P * k:P * (k + 1), :], d32[:])
        att_cm.__exit__(None, None, None)
        xp_cm.__exit__(None, None, None)
        return

    # ---------- stage 4: attention (12 heads, own 512 queries) ----------
    # o_raw[:, h, :] holds [65, 512] (64 feats + sum row 64)
    o_raw = attp.tile([HD + 1, NH, 512], BF, name="o_raw")
    for hp in range(NH // 2):
        kf = hp
        ps_os = [psum_o.tile([HD + 1, 512], F32, name="po") for _ in range(2)]
        for kt in range(TT16):
            pss = []
            for sub in range(2):   # both row-groups back-to-back: concurrent
                off = HD * sub
                ps_s = psum_big.tile([P, 512], F32, name="pbig")
                nc.tensor.matmul(ps_s[:],
                                 KTs[off:off + HD, kf, P * kt:P * (kt + 1)],
                                 QTs[off:off + HD, kf, :],
                                 start=True, stop=True)
                pss.append(ps_s)
            for sub in range(2):
                e_t = eP.tile([P, 512], F8, name="e")
                nc.scalar.activation(e_t[:], pss[sub][:], AF.Exp, scale=SCALE)
                nc.tensor.matmul(ps_os[sub][:], V_aug[:, kt, 2 * hp + sub, :],
                                 e_t[:], start=(kt == 0), stop=(kt == TT16 - 1))
        for sub in range(2):
            nc.vector.tensor_copy(o_raw[:, 2 * hp + sub, :], ps_os[sub][:])
    # batched softmax normalization (sum rows live on partition 64)
    sums = small.tile([NH, 512], BF, name="sums", bufs=1)
    nc.sync.dma_start(sums[:], o_raw[HD:HD + 1, :, :])
    recb = small.tile([NH, 512], BF, name="recb", bufs=1)
    with nc.allow_low_precision(reason="softmax norm factor in bf16"):
        nc.vector.reciprocal(recb[:], sums[:])
    rec_d = dram.tile([NH, 512], BF)
    nc.sync.dma_start(rec_d[:], recb[:])
    rec_bc = attp.tile([HD, NH, 512], BF, name="rec_bc")
    nc.sync.dma_start(rec_bc[:], _rep(rec_d[:], HD))
    oT = xp.tile([P, KT6, TOK], BF, name="oT")
    for h in range(NH):
        kf = h // 2
        off = HD * (h % 2)
        if off == 0:
            nc.vector.tensor_mul(oT[0:HD, kf, :], o_raw[0:HD, h, :],
                                 rec_bc[:, h, :])
        else:
            o_n = wrk.tile([P, 512], BF, name="ost", bufs=2)[0:HD, :]
            nc.vector.tensor_mul(o_n, o_raw[0:HD, h, :], rec_bc[:, h, :])
            nc.sync.dma_start(oT[off:off + HD, kf, :], o_n)
    att_cm.__exit__(None, None, None)

    if STAGE < 5:
        for k in range(KT6):
            d32 = wrk.tile([P, 512], F32, name="w512")
            nc.vector.tensor_copy(d32[:], oT[:, k, :])
            nc.sync.dma_start(io["out"][P * k:P * (k + 1), :], d32[:])
        xp_cm.__exit__(None, None, None)
        return

    # ---------- stage 5: mffn (own 512 tokens) ----------
    gT_cm = tc.tile_pool(name="gTp", bufs=1)
    gTp = gT_cm.__enter__()
    wk_cm = tc.tile_pool(name="wk", bufs=1)
    wk = wk_cm.__enter__()
    xoT = xp.tile([P, KT6, TOK], F32, name="xoT")
    nc.sync.dma_start(xoT[:], io["xoT"].rearrange("(k p) t -> p k t", p=P))

    def mlp1(inT, w1_dram, gT):
        w1sb = wk.tile([P, KT6, FF], BF, name="wmlp", bufs=1)
        for k in range(KT6):
            nc.sync.dma_start(w1sb[:, k, :], w1_dram[P * k:P * (k + 1), :])
        for m in range(FFT):
            ps = psum_big.tile([P, 512], F32, name="pbig")
            for k in range(KT6):
                nc.tensor.matmul(ps[:], w1sb[:, k, P * m:P * (m + 1)],
                                 inT[:, k, :], start=(k == 0), stop=(k == KT6 - 1))
            nc.scalar.activation(gT[:, m, :], ps[:], AF.Tanh if SIM_SAFE else AF.Gelu)

    def mlp2_T(gT, w2_dram, ac, res_T, out_T):
        # out_T[:, f, :] = res_T[:, f, :] + ac[f] * (w2.T @ g)  (transposed out)
        w2sb = wk.tile([P, FFT, H], BF, name="wmlp", bufs=1)
        for k in range(FFT):
            nc.sync.dma_start(w2sb[:, k, :], w2_dram[P * k:P * (k + 1), :])
        for f in range(KT6):
            ps = psum_big.tile([P, 512], F32, name="pbig")
            for k in range(FFT):
                nc.tensor.matmul(ps[:], w2sb[:, k, P * f:P * (f + 1)],
                                 gT[:, k, :], start=(k == 0), stop=(k == FFT - 1))
            nc.vector.scalar_tensor_tensor(out_T[:, f, :], ps[:], ac[:, f:f + 1],
                                           res_T[:, f, :], ALU.mult, ALU.add)

    gT = gTp.tile([P, FFT, TOK], BF, name="gT")
    mlp1(oT, io["wm1"], gT)
    x1T = xp.tile([P, KT6, TOK], F32, name="x1T")
    mlp2_T(gT, io["wm2"], A1c, xoT, x1T)

    if STAGE < 6:
        for k in range(KT6):
            nc.sync.dma_start(io["out"][P * k:P * (k + 1), :], x1T[:, k, :])
        wk_cm.__exit__(None, None, None)
        gT_cm.__exit__(None, None, None)
        xp_cm.__exit__(None, None, None)
        return

    # ---------- stage 6: LN2 (transposed) ----------
    x1b = xp.tile([P, KT6, TOK], BF, name="x1b")
    ps_mu = psum_big.tile([P, 512], F32, name="pbig")
    ps_sq = psum_big.tile([P, 512], F32, name="pbig")
    for k in range(KT6):
        nc.vector.tensor_copy(x1b[:, k, :], x1T[:, k, :])
        xsq = wrk.tile([P, 512], BF, name="xsqb", bufs=2)
        nc.vector.tensor_mul(xsq[:], x1b[:, k, :], x1b[:, k, :])
        nc.tensor.matmul(ps_mu[:], ones_bf[:], x1b[:, k, :],
                         start=(k == 0), stop=(k == KT6 - 1))
        nc.tensor.matmul(ps_sq[:], ones_bf[:], xsq[:],
                         start=(k == 0), stop=(k == KT6 - 1))
    mu = wrk.tile([P, 512], F32, name="w512")
    nc.vector.tensor_scalar(mu[:], ps_mu[:], 1.0 / H, None, ALU.mult)
    musq = wrk.tile([P, 512], F32, name="w512")
    nc.vector.tensor_mul(musq[:], mu[:], mu[:])
    varme = wrk.tile([P, 512], F32, name="w512")
    nc.vector.scalar_tensor_tensor(varme[:], ps_sq[:], 1.0 / H, musq[:],
                                   ALU.mult, ALU.subtract)
    std = wrk.tile([P, 512], F32, name="w512")
    nc.scalar.activation(std[:], varme[:], AF.Sqrt, bias=eps_ap[:])
    c1 = wrk.tile([P, 512], F32, name="c1ln2")
    nc.vector.reciprocal(c1[:], std[:])
    c0 = wrk.tile([P, 512], F32, name="c0ln2")
    nc.vector.tensor_mul(c0[:], mu[:], c1[:])
    h2T = xp.tile([P, KT6, TOK], BF, name="h2T")
    for k in range(KT6):
        xn = wrk.tile([P, 512], F32, name="w512")
        nc.vector.tensor_mul(xn[:], x1T[:, k, :], c1[:])
        nc.vector.tensor_sub(xn[:], xn[:], c0[:])
        nc.vector.tensor_scalar(h2T[:, k, :], xn[:],
                                G2c[:, k:k + 1], B2c[:, k:k + 1],
                                ALU.mult, ALU.add)

    if STAGE < 7:
        for k in range(KT6):
            nc.sync.dma_start(io["out"][P * k:P * (k + 1), :], x1T[:, k, :])
        wk_cm.__exit__(None, None, None)
        gT_cm.__exit__(None, None, None)
        xp_cm.__exit__(None, None, None)
        return

    # ---------- stage 7: FFN ----------
    gT2 = gTp.tile([P, FFT, TOK], BF, name="gT")
    mlp1(h2T, io["wf1"], gT2)
    outT = xp.tile([P, KT6, TOK], F32, name="outT")
    mlp2_T(gT2, io["wf2"], A2c, x1T, outT)
    for k in range(KT6):
        nc.sync.dma_start(io["out"][P * k:P * (k + 1), :], outT[:, k, :])

    wk_cm.__exit__(None, None, None)
    gT_cm.__exit__(None, None, None)
    xp_cm.__exit__(None, None, None)


_CACHE = {}


def _build():
    key = (STAGE, SIM_SAFE)
    if key in _CACHE:
        return _CACHE[key]
    nc = bacc.Bacc("TRN2", target_bir_lowering=False, debug=False, num_devices=N_CORES)
    io = {}
    def inp(name, shape, dt):
        io[name] = nc.dram_tensor(name, shape, dt, kind="ExternalInput").ap()
    inp("xT", [H, TOK], BF)
    inp("xoT", [H, TOK], F32)
    inp("tT", [H, 1], BF)
    inp("wqkv", [H, 3 * H], BF)
    inp("wm1", [H, FF], BF)
    inp("wm2", [FF, H], BF)
    inp("wf1", [H, FF], BF)
    inp("wf2", [FF, H], BF)
    inp("ss1", [H, SS], BF)
    inp("ss2s", [SS, SSH], F8)
    inp("ln1g_c", [P, KT6], F32)
    inp("ln1b_c", [P, KT6], F32)
    inp("ln2g_c", [P, KT6], F32)
    inp("ln2b_c", [P, KT6], F32)
    io["out"] = nc.dram_tensor("out", [H, TOK], F32, kind="ExternalOutput").ap()
    from contextlib import ExitStack
    with tile.TileContext(nc) as tc, ExitStack() as ctx:
        _emit(ctx, tc, io)
    nc.compile()
    _CACHE[key] = nc
    return nc


def _bf16(a):
    return np.ascontiguousarray(a.astype(ml_dtypes.bfloat16))


def _f8(a):
    return np.ascontiguousarray(np.asarray(a, np.float32).astype(ml_dtypes.float8_e4m3))


def _cols(v):
    return np.ascontiguousarray(np.asarray(v, np.float32).reshape(KT6, P).T)


def make_in_maps(inputs):
    x = np.asarray(inputs["x"], np.float32)
    t = np.asarray(inputs["t"], np.float32)
    for zname in ("b_qkv", "b_mffn1", "b_mffn2", "b_ss1", "b_ss2", "b_ffn1", "b_ffn2"):
        if np.any(np.asarray(inputs[zname])):
            raise NotImplementedError(f"{zname} must be zero (kernel folds biases away)")

    wqkv = _bf16(inputs["w_qkv"])
    wm1 = _bf16(inputs["w_mffn1"])
    wm2 = _bf16(inputs["w_mffn2"])
    wf1 = _bf16(inputs["w_ffn1"])
    wf2 = _bf16(inputs["w_ffn2"])
    ss1 = np.asarray(inputs["w_ss1"], np.float32)
    ss2 = np.asarray(inputs["w_ss2"], np.float32)
    t2T = _bf16(t.reshape(2, H).T)
    ss1_bf = _bf16(ss1)
    ln = {f"ln{i}{gb}_c": _cols(inputs[f"ln{i}_{gb}"])
          for i in (1, 2) for gb in ("g", "b")}

    in_maps = []
    for c in range(N_CORES):
        b, j = divmod(c, 4)
        xo = x[b, TOK * j:TOK * (j + 1)]
        in_maps.append({
            "xT": _bf16(xo.T),
            "xoT": np.ascontiguousarray(xo.T),
            "tT": np.ascontiguousarray(t2T[:, b:b + 1]),
            "wqkv": wqkv,
            "wm1": wm1, "wm2": wm2, "wf1": wf1, "wf2": wf2,
            "ss1": ss1_bf,
            "ss2s": _f8(64.0 * ss2[:, SSH * j:SSH * (j + 1)]),
            **ln,
        })
    return in_maps


def kernel(**inputs):
    in_maps = make_in_maps(inputs)
    nc = _build()
    res = run_bass_kernel_spmd(nc, in_maps, core_ids=list(range(N_CORES)))
    out = np.empty((B, T, H), np.float32)
    for c in range(N_CORES):
        b, j = divmod(c, 4)
        out[b, TOK * j:TOK * (j + 1)] = res.results[c]["out"].T
    return out
